# revision 42
# baseline (speedup 1.0000x reference)
"""BitLinear (4-bit activation quant + ternary weight) Trainium2 kernel.

Full computation:
    xq  = round(clip(x / max_abs(x, row) * 7)) * max_abs / 7      (per-row 4-bit quant)
    wq  = sign_thresholded(w) * mean_abs(w, row)                   (ternary weight)
    out = xq @ wq.T + bias
outputs f32; rel-err budget 2e-2.

Strategy (8 NeuronCores, data-parallel over rows of x):
  - Shard x rows 8 ways; replicate weight.
  - HBM traffic is the roofline: x, w are shipped in fp16 and the output is
    returned in fp16 (upcast on host). Measured end-to-end rel err ~1.0e-2
    vs the 2e-2 gate (flips of the 4-bit quantization from fp16 rounding of
    x dominate; fp16 w adds threshold flips; fp16 out adds 2e-4).
  - On-chip, the matmul runs on exact small integers in fp8 (q in [-7,7],
    sign in {-1,0,1}) with DoubleRow perf mode, so the PE accumulation is
    exact; the row scale (max_abs/7) and column scale (alpha) are applied to
    the f32 PSUM output in one fused scalar_tensor_tensor eviction.
  - Rounding uses the fp16 magic-number trick (+1.5*2^10, round-half-even
    on the f32->fp16 output cast), which keeps t in fp16 so the PE
    transposes run at 1 cycle/row and the DVE quant runs in 4x mode.
  - Engine balance: DVE does the max|x| reduces (no 16-bit fast path
    exists for reduce) plus half the PSUM out-evictions; ACT does the fp8
    qt eviction, the |w| row-sums (Abs + accum_out) and the other half of
    the out-evictions (row-scale, with the fp16 column-scale multiply on
    DVE in 2x mode); Pool (no PSUM access) owns the x quant multiply-add;
    PE does fp16 transposes + fp8 DoubleRow matmuls.
  - Emission order software-pipelines three in-order sequencer streams:
    matmul+evict (LEAD=7 stiles behind), PE transposes (TLAG=4 behind),
    and the quant front, so no sequencer head-of-line blocks on a
    same-iteration dependency; out DMAs ship a deferred 1 MiB quad on the
    SP ring so their semaphore waits never stall a compute sequencer.
"""

import os
import sys

os.environ.setdefault("MYCRO_LOCAL_CACHE", "1")

for _p in ("/opt/trn_rl_repo", "/root/.axon_site/_ro/trn_rl_repo"):
    if os.path.isdir(_p) and _p not in sys.path:
        sys.path.insert(0, _p)

import numpy as np

N_CORES = 8
S_SHARD = 4096  # rows of x per core (8*4096 total / 8 cores)
IN_F = 1024
OUT_F = 1024
P = 128  # partitions
N_STILES = S_SHARD // P  # 32
N_KTILES = IN_F // P  # 8
N_OTILES = OUT_F // P  # 8
MM_N = 512  # matmul moving free dim (one PSUM bank of f32)
N_OHALF = OUT_F // MM_N  # 2

MAGIC = 1536.0  # 1.5 * 2**10: fp16 round-to-nearest-even on the output cast
EPS = 1e-06

_prog_cache = {}


def _build_program(with_bias: bool):
    KTQDVE = int(os.environ.get("KTQDVE", "8"))
    KFUSE = int(os.environ.get("KFUSE", "2"))
    KTMPSPLIT = int(os.environ.get("KTMPSPLIT", "0"))
    KWLOAD = int(os.environ.get("KWLOAD", "3"))
    KSGNP = int(os.environ.get("KSGNP", "0"))
    KTAILH = int(os.environ.get("KTAILH", "0"))
    KTQACT = int(os.environ.get("KTQACT", "0"))
    import concourse.bass as bass
    import concourse.mybir as mybir
    import concourse.tile as tile
    from concourse import bacc, bass_isa
    from concourse.masks import make_identity

    f32 = mybir.dt.float32
    f16 = mybir.dt.float16
    bf16 = mybir.dt.bfloat16
    f8 = mybir.dt.float8e4
    Alu = mybir.AluOpType
    Act = mybir.ActivationFunctionType

    nc = bacc.Bacc("TRN2", target_bir_lowering=False, debug=False)

    x_in = nc.dram_tensor("x_shard", [S_SHARD, IN_F], f16, kind="ExternalInput")
    w_in = nc.dram_tensor("weight", [OUT_F, IN_F], f16, kind="ExternalInput")
    if with_bias:
        b_in = nc.dram_tensor("bias", [OUT_F], f32, kind="ExternalInput")
    out_d = nc.dram_tensor("out", [S_SHARD, OUT_F], f16, kind="ExternalOutput")

    with tile.TileContext(nc) as tc:
        from contextlib import ExitStack as _ES

        _wstack = _ES()
        with (
            tc.tile_pool(name="singles", bufs=1) as singles,
            tc.tile_pool(name="wtmp", bufs=2) as wtmp,
            tc.tile_pool(name="signp", bufs=2) as signp,
            tc.tile_pool(name="xp", bufs=3) as xp,
            tc.tile_pool(name="tp", bufs=6) as tp,
            tc.tile_pool(name="qtp", bufs=N_STILES + 1) as qtp,
            tc.tile_pool(name="outp", bufs=3) as outp,
            tc.tile_pool(name="stats", bufs=8) as stats,
            tc.tile_pool(name="ma7p", bufs=N_STILES + 1) as ma7p,
            tc.tile_pool(name="tpsum", bufs=int(os.environ.get("KTPS", "2")), space="PSUM") as tpsum,
            tc.tile_pool(name="mpsum", bufs=int(os.environ.get("KMPS", "3")), space="PSUM") as mpsum,
            tc.tile_pool(name="dramp", bufs=1, space="DRAM") as dramp,
        ):
            # ---------------- one-time setup ----------------
            identity16 = singles.tile([P, P], f16)
            make_identity(nc, identity16)

            magneg = singles.tile([P, 1], f32)
            nc.vector.memset(magneg, -MAGIC)
            magpos = singles.tile([P, 1], f32)
            nc.vector.memset(magpos, MAGIC)

            # signT8[i_sub, k, o] = ternarized sign of weight[o, k*128+i_sub]
            # fp8 for DoubleRow matmuls (values {-1,0,1}: exact)
            signT8 = singles.tile([P, N_KTILES, OUT_F], f8)
            alpha_raw = singles.tile([P, N_OTILES], f32)  # row sums of |w|

            wpool = _wstack.enter_context(tc.tile_pool(name="wpool", bufs=8))
            w_tiles = []
            for j in range(N_OTILES):
                w_t = wpool.tile([P, IN_F], f16, tag="w")
                w_tiles.append(w_t)
                # odd tiles ride the scalar ring immediately; even tiles are
                # issued inside the prologue, interleaved behind the first x
                # loads so the SP ring serves the quant pipeline first
                if j % 2:
                    nc.scalar.dma_start(out=w_t, in_=w_in[j * P : (j + 1) * P, :])

            def emit_wload_even(js):
                for j in js:
                    if j < N_OTILES:
                        nc.sync.dma_start(
                            out=w_tiles[j], in_=w_in[j * P : (j + 1) * P, :]
                        )

            def emit_wabs(j):
                # |w| row sums on ACT: Abs with free-dim accumulator; the
                # elementwise |w| output is a discarded scratch tile
                scratch = wtmp.tile([P, IN_F], f16, tag="wabs")
                nc.scalar.activation(
                    out=scratch,
                    in_=w_tiles[j],
                    func=Act.Abs,
                    accum_out=alpha_raw[:, j : j + 1],
                )

            # ---- quant prologue: first few s-tiles' quant+transpose, so the
            # PE has ready work while the weight ternarization chain resolves.
            # x arrives in growing groups (single tiles first for a fast
            # pipeline start, then 1 MiB quads to keep the DMA-issue count —
            # and with it sequencer ring stalls — low).
            X_GROUPS = {0: 1, 1: 1, 2: 2} | {s: 4 for s in range(4, N_STILES, 4)}
            x_start = {}
            for _gs, _gn in X_GROUPS.items():
                for _i in range(_gn):
                    x_start[_gs + _i] = _gs
            x_tiles = {}

            def emit_quant(s):
                g0 = x_start[s]
                if s == g0:
                    gn = X_GROUPS[g0]
                    xg = xp.tile([P, gn, IN_F], f16, tag=f"x{gn}")
                    if gn == 1:
                        nc.sync.dma_start(
                            out=xg[:, 0, :], in_=x_in[s * P : (s + 1) * P, :]
                        )
                    else:
                        nc.sync.dma_start(
                            out=xg,
                            in_=x_in[s * P : (s + gn) * P, :].rearrange(
                                "(g p) f -> p g f", p=P
                            ),
                        )
                    x_tiles[g0] = xg
                x_t = x_tiles[g0][:, s - g0, :]
                ma = stats.tile([P, 1], f32, tag="ma")
                nc.vector.tensor_reduce(
                    out=ma,
                    in_=x_t,
                    axis=mybir.AxisListType.X,
                    op=Alu.max,
                    apply_absolute_value=True,
                )
                # row scale = max(ma, EPS)/7 ; inv = 7/max(ma, EPS)
                ma7 = ma7p.tile([P, 1], f32, tag="ma7")
                nc.vector.tensor_scalar(
                    out=ma7,
                    in0=ma,
                    scalar1=float(1.0 / 7.0),
                    scalar2=float(EPS / 7.0),
                    op0=Alu.mult,
                    op1=Alu.max,
                )
                inv = stats.tile([P, 1], f32, tag="inv")
                nc.vector.reciprocal(out=inv, in_=ma7)
                # t = fp16(x*inv + 1536): the f32->fp16 output cast rounds the
                # fraction half-to-even (t in [1529,1543], fp16 ulp there = 1).
                # Mostly on Pool (GPSIMD cannot touch PSUM, so this SBUF-only
                # stream is the big one it can own); the first tiles run on
                # DVE (4x fp16 mode, 327ns) to keep Pool clear for the weight
                # ternarization chain during warmup.
                t_t = tp.tile([P, IN_F], f16, tag="t")
                if KTQDVE <= s < KTQDVE + KTQACT:
                    nc.scalar.activation(
                        out=t_t, in_=x_t, func=Act.Identity, scale=inv, bias=magpos
                    )
                else:
                    quant_eng = nc.vector if s < KTQDVE else nc.gpsimd
                    quant_eng.tensor_scalar(
                        out=t_t,
                        in0=x_t,
                        scalar1=inv,
                        scalar2=MAGIC,
                        op0=Alu.mult,
                        op1=Alu.add,
                    )
                return ma7, t_t

            def emit_transpose(s, ma7, t_t):
                # transpose t into [i, s] layout via PE (8 blocks, one psum
                # tile; fp16 transposes run at 1 cycle/row vs 2 for f32).
                # Emitted a couple of iterations after the quant so the PE's
                # in-order queue never holds a transpose that waits on the
                # Pool quant ahead of a ready matmul.
                qt_ps = tpsum.tile([P, IN_F], f16, tag="tps16")
                for k in range(N_KTILES):
                    nc.tensor.transpose(
                        qt_ps[:, k * P : (k + 1) * P],
                        t_t[:, k * P : (k + 1) * P],
                        identity16,
                    )
                # evict with fused -MAGIC subtract + fp8 cast (exact ints)
                qt_sb = qtp.tile([P, N_KTILES, P], f8, tag="qt")
                nc.scalar.activation(
                    out=qt_sb.rearrange("p k c -> p (k c)"),
                    in_=qt_ps,
                    func=Act.Identity,
                    bias=magneg,
                    scale=1.0,
                )
                return ma7, qt_sb

            out_groups = {}

            def emit_matmul(s, ma7, qt_sb):
                # output rows leave four s-tiles per 1 MiB DMA, except the
                # last four which ship individually so the tail drains fast
                if s >= N_STILES - 4:
                    out_sb = outp.tile([P, OUT_F], f16, tag="otail")
                elif s % 4 == 0:
                    out4 = outp.tile([P, 4, OUT_F], f16, tag="o")
                    out_groups[s] = out4
                    out_sb = out4[:, 0, :]
                else:
                    out_sb = out_groups[s - s % 4][:, s % 4, :]
                ps = mpsum.tile([P, OUT_F], f32, tag="mm")
                for h in range(N_OHALF):
                    for t in range(N_KTILES // 2):
                        nc.tensor.matmul(
                            ps[:, h * MM_N : (h + 1) * MM_N],
                            lhsT=qt_sb[:, 2 * t : 2 * t + 2, :],
                            rhs=signT8[
                                :, 2 * t : 2 * t + 2, h * MM_N : (h + 1) * MM_N
                            ],
                            start=(t == 0),
                            stop=(t == N_KTILES // 2 - 1),
                            perf_mode=mybir.MatmulPerfMode.DoubleRow,
                        )
                # out = (S * rowscale) * colscale.  Only DVE and ACT can read
                # PSUM; alternate between a fused DVE stt eviction and an
                # ACT(row-scale ptr) + DVE(fp16 2x colscale mult) pair to
                # split the eviction load across both engines.  In the drain
                # tail the eviction runs per 512-wide half so it overlaps the
                # second half's matmul accumulation.
                halves = (
            		[(0, MM_N), (MM_N, OUT_F)] if s >= N_STILES - KTAILH else [(0, OUT_F)]
                )
                for lo, hi in halves:
                    if s % KFUSE != KFUSE - 1:
                        nc.scalar.activation(
                            out=out_sb[:, lo:hi],
                            in_=ps[:, lo:hi],
                            func=Act.Identity,
                            scale=ma7,
                        )
                        nc.vector.tensor_tensor(
                            out=out_sb[:, lo:hi],
                            in0=out_sb[:, lo:hi],
                            in1=colb16[:, lo:hi],
                            op=Alu.mult,
                        )
                    else:
                        nc.vector.scalar_tensor_tensor(
                            out=out_sb[:, lo:hi],
                            in0=ps[:, lo:hi],
                            scalar=ma7,
                            in1=colb16[:, lo:hi],
                            op0=Alu.mult,
                            op1=Alu.mult,
                        )
                if with_bias:
                    nc.gpsimd.tensor_tensor(
                        out=out_sb, in0=out_sb, in1=biasb, op=Alu.add
                    )
                if s >= N_STILES - 4:
                    nc.scalar.dma_start(
                        out=out_d[s * P : (s + 1) * P, :], in_=out_sb
                    )
                elif s % 4 == 3:
                    # queue the quad; it is shipped at the next iteration so
                    # its eviction deps are resolved by the time the SP
                    # sequencer parks on the DMA's waits (waits hold the SEQ)
                    pending_out.append(
                        (
                            out_d[(s - 3) * P : (s + 1) * P, :].rearrange(
                                "(g p) f -> p g f", p=P
                            ),
                            out_groups.pop(s - 3),
                        )
                    )

            def flush_out():
                while pending_out:
                    dst, src = pending_out.pop(0)
                    nc.sync.dma_start(out=dst, in_=src)

            def emit_wprep_tail():
                # global threshold = 0.05 * mean(|w|)
                g0 = stats.tile([P, 1], f32, tag="g0")
                nc.vector.tensor_reduce(
                    out=g0, in_=alpha_raw, axis=mybir.AxisListType.X, op=Alu.add
                )
                g1 = stats.tile([P, 1], f32, tag="g1")
                nc.gpsimd.partition_all_reduce(
                    out_ap=g1, in_ap=g0, channels=P, reduce_op=bass_isa.ReduceOp.add
                )
                nc.vector.tensor_scalar(
                    out=thr,
                    in0=g1,
                    scalar1=float(0.05 / (OUT_F * IN_F)),
                    scalar2=None,
                    op0=Alu.mult,
                )
                nc.vector.tensor_scalar(
                    out=nthr, in0=thr, scalar1=-1.0, scalar2=None, op0=Alu.mult
                )
                # alpha[o] = rowsum / IN_F (fp16: its ~2e-4 rounding is far
                # inside the error budget and halves the broadcast DMA)
                nc.vector.tensor_scalar(
                    out=alpha_sb,
                    in0=alpha_raw,
                    scalar1=float(1.0 / IN_F),
                    scalar2=None,
                    op0=Alu.mult,
                )

                # ternary sign: sign = (w >= thr) + (w > -thr) - 1; the first
                # compare alternates GPSIMD/DVE (the serial 8-tile chain gates
                # the first matmul), the second runs on DVE
                for j in range(N_OTILES):
                    tmp = wtmp.tile([P, IN_F], bf16, tag="tmp")
                    tmp_eng = nc.gpsimd if (j % 2 == 0 or not KTMPSPLIT) else nc.vector
                    tmp_eng.tensor_scalar(
                        out=tmp,
                        in0=w_tiles[j],
                        scalar1=nthr,
                        scalar2=-1.0,
                        op0=Alu.is_gt,
                        op1=Alu.add,
                    )
                    sgn = signp.tile([P, IN_F], f16, tag="sgn")
                    sgn_eng = nc.gpsimd if (KSGNP and j % 2 == 1) else nc.vector
                    sgn_eng.scalar_tensor_tensor(
                        out=sgn,
                        in0=w_tiles[j],
                        scalar=thr,
                        in1=tmp,
                        op0=Alu.is_ge,
                        op1=Alu.add,
                    )
                    # transpose 8x [128,128] blocks into one PSUM bank, evict.
                    # Same fp16 dtype + pool tag as the t transposes so the
                    # pool rotates through 4 single-bank tiles.
                    ps = tpsum.tile([P, IN_F], f16, tag="tps16")
                    for k in range(N_KTILES):
                        nc.tensor.transpose(
                            ps[:, k * P : (k + 1) * P],
                            sgn[:, k * P : (k + 1) * P],
                            identity16,
                        )
                    nc.scalar.activation(
                        out=signT8[:, :, j * P : (j + 1) * P],
                        in_=ps.rearrange("p (k c) -> p k c", k=N_KTILES),
                        func=Act.Copy,
                    )

                # column scale alpha broadcast to all partitions via DRAM bounce
                nc.sync.dma_start(
                    out=alpha_dram.rearrange("j p -> p j"), in_=alpha_sb
                )
                alpha_flat = alpha_dram.rearrange("j p -> (j p)")
                bcast_src = bass.AP(
                    tensor=alpha_flat.tensor,
                    offset=alpha_flat.offset,
                    ap=[[0, P]] + list(alpha_flat.ap),
                )
                nc.sync.dma_start(out=colb16, in_=bcast_src)

                if with_bias:
                    bias_src = bass.AP(
                        tensor=b_in.tensor
                        if hasattr(b_in, "tensor")
                        else b_in[:].tensor,
                        offset=b_in[:].offset,
                        ap=[[0, P]] + list(b_in[:].ap),
                    )
                    nc.sync.dma_start(out=biasb, in_=bias_src)

            thr = singles.tile([P, 1], f32)
            nthr = singles.tile([P, 1], f32)
            alpha_sb = singles.tile([P, N_OTILES], f16)
            alpha_dram = dramp.tile([N_OTILES, P], f16)
            colb16 = singles.tile([P, OUT_F], f16)
            biasb = None
            if with_bias:
                biasb = singles.tile([P, OUT_F], f32, tag="biasb")

            # Phase 1: quantize + transpose ALL s-tiles (PE does transposes
            # while the weight-ternarization chain resolves); |w| row-sums
            # interleave into the ACT stream between the early evictions, and
            # the full sign chain is emitted early (after s=4) so it sits near
            # the front of each engine's FIFO.
            LEAD = min(int(os.environ.get("KLEAD", "7")), N_STILES - 1)
            # |w| row-sum pairs finish by s=3, the sign chain is emitted at
            # WPREP_S, and the first matmul emission is clamped to come after
            # it: a matmul emitted before the signT8 writes would read the
            # uninitialized tile (Tile deps follow program order).
            WPREP_S = min(N_OTILES // 2, N_STILES - 1)
            LEAD = max(LEAD, WPREP_S + 1)
            TLAG = min(int(os.environ.get("KTLAG", "4")), LEAD - 1)
            pending_out = []
            prologue = []
            staged = []
            for s in range(N_STILES):
                # emit the LEAD-delayed matmul/eviction FIRST: its deps are
                # old, so each in-order sequencer dispatches it without
                # stalling before starting stile s's fresh quant chain
                if s > LEAD:
                    flush_out()
                    emit_matmul(s - LEAD - 1, *staged[s - LEAD - 1])
                prologue.append(emit_quant(s))
                if s >= TLAG:
                    staged.append(emit_transpose(s - TLAG, *prologue[s - TLAG]))
                # even w tiles ride the SP ring between the early x loads (x
                # first: the max-reduce stream paces the whole warmup); odd
                # tiles were issued on the scalar ring at program start, so
                # their |w| sums run first while the even halves land
                if KWLOAD == 0:
                    if s == 0:
                        emit_wload_even((0, 2, 4, 6))
                elif KWLOAD == 1:
                    if s == 1:
                        emit_wload_even((0, 2))
                    elif s == 3:
                        emit_wload_even((4, 6))
                elif KWLOAD == 3:
                    if s == 1:
                        emit_wload_even((0, 2))
                    elif s == 4:
                        emit_wload_even((4, 6))
                elif s == 4:
                    emit_wload_even((0, 2, 4, 6))
                # |w| sums must be emitted after their tile's DMA; with
                # KWLOAD=3 the w4/w6 loads land at s=4, so their sums (and
                # nothing else) run there, right before the ternarization
                if KWLOAD == 3:
                    _wabs_sched = {0: (1, 3), 1: (5, 7), 2: (0, 2), 4: (4, 6)}
                else:
                    _wabs_sched = {0: (1, 3), 1: (5, 7), 2: (0, 2), 3: (4, 6)}
                for j in _wabs_sched.get(s, ()):
                    emit_wabs(j)
                if s == WPREP_S:
                    emit_wprep_tail()
                    w_tiles.clear()
                    _wstack.close()  # releases the weight pool
            for s in range(N_STILES, N_STILES + LEAD + 1):
                if s > LEAD:
                    flush_out()
                    emit_matmul(s - LEAD - 1, *staged[s - LEAD - 1])
                if N_STILES > s - TLAG >= len(staged):
                    staged.append(emit_transpose(s - TLAG, *prologue[s - TLAG]))
            flush_out()

    nc.compile()
    return nc


def _get_program(with_bias: bool):
    key = bool(with_bias)
    if key not in _prog_cache:
        _prog_cache[key] = _build_program(key)
    return _prog_cache[key]


def _make_in_maps(x: np.ndarray, weight: np.ndarray, bias: np.ndarray):
    xf = x.astype(np.float16).reshape(-1, IN_F)
    w = np.ascontiguousarray(weight.astype(np.float16))
    b = np.ascontiguousarray(bias.astype(np.float32, copy=False))
    with_bias = bool(np.any(b != 0.0))
    in_maps = []
    for c in range(N_CORES):
        m = {
            "x_shard": np.ascontiguousarray(xf[c * S_SHARD : (c + 1) * S_SHARD]),
            "weight": w,
        }
        if with_bias:
            m["bias"] = b
        in_maps.append(m)
    return in_maps, with_bias


def kernel(x: np.ndarray, weight: np.ndarray, bias: np.ndarray) -> np.ndarray:
    from concourse.bass_utils import run_bass_kernel_spmd

    B, S, in_f = x.shape
    out_f = weight.shape[0]
    assert in_f == IN_F and out_f == OUT_F and B * S == N_CORES * S_SHARD

    in_maps, with_bias = _make_in_maps(x, weight, bias)
    nc = _get_program(with_bias)

    res = run_bass_kernel_spmd(nc, in_maps, core_ids=list(range(N_CORES)))
    out = np.concatenate([res.results[c]["out"] for c in range(N_CORES)], axis=0)
    return out.reshape(B, S, OUT_F).astype(np.float32)
